# revision 1
# baseline (speedup 1.0000x reference)
"""DGCNN forward on 8 Trainium2 NeuronCores.

Sharding: data-parallel over the batch/graph dim — the 8 graphs in the
batch map 1:1 onto the 8 NeuronCores. kNN graph construction, the
EdgeConv MLPs and the per-graph max-pool are fully independent per
graph; weights are replicated. BatchNorm runs in training mode in the
reference (batch statistics over the full node/edge dim across ALL
graphs), so the per-core partial sums / sums-of-squares are allreduced
(lax.psum) across the 8 cores at each of the 9 BN sites.

The whole forward is one jitted shard_map program: XLA compiles it to a
single SPMD NEFF per core; cross-core traffic is only the tiny BN
statistic allreduces ([1,64]..[1,1024] vectors).
"""
import numpy as np

K = 10
EPS = 1e-5
N_CORES = 8


def _forward_sharded(x, w, B):
    """Per-core body under shard_map: x is this core's shard (one graph,
    [N, 6]). BN stats are completed with psum over the 'c' mesh axis."""
    import jax
    import jax.numpy as jnp
    from jax import lax

    def bn(h, g, b):
        # training-mode batch-norm: statistics over all rows on all cores
        n = h.shape[0] * N_CORES
        s = lax.psum(jnp.sum(h, axis=0, keepdims=True), 'c')
        sq = lax.psum(jnp.sum(h * h, axis=0, keepdims=True), 'c')
        m = s / n
        v = sq / n - m * m
        return (h - m) * lax.rsqrt(v + EPS) * g + b

    def edge_conv(xi, w0, b0, g0, be0, w1, b1, g1, be1):
        N, D = xi.shape
        sq = jnp.sum(xi * xi, axis=-1)
        d2 = sq[:, None] + sq[None, :] - 2.0 * (xi @ xi.T)
        _, idx = lax.top_k(-d2, K)            # self included (d2=0)
        neigh = xi[idx]                       # [N,K,D]
        xc = jnp.broadcast_to(xi[:, None, :], neigh.shape)
        e = jnp.concatenate([xc, neigh - xc], axis=-1).reshape(N * K, 2 * D)
        h = bn(jax.nn.relu(e @ w0 + b0), g0, be0)
        h = bn(jax.nn.relu(h @ w1 + b1), g1, be1)
        return h.reshape(N, K, -1).max(axis=1)

    x1 = edge_conv(x, w['c1_w0'], w['c1_b0'], w['c1_g0'], w['c1_be0'],
                   w['c1_w1'], w['c1_b1'], w['c1_g1'], w['c1_be1'])
    x2 = edge_conv(x1, w['c2_w0'], w['c2_b0'], w['c2_g0'], w['c2_be0'],
                   w['c2_w1'], w['c2_b1'], w['c2_g1'], w['c2_be1'])
    x3 = edge_conv(x2, w['c3_w0'], w['c3_b0'], w['c3_g0'], w['c3_be0'],
                   w['c3_w1'], w['c3_b1'], w['c3_g1'], w['c3_be1'])
    cat = jnp.concatenate([x1, x2, x3], axis=1)
    x4 = bn(jax.nn.relu(cat @ w['l1_w'] + w['l1_b']), w['l1_g'], w['l1_be'])
    pooled = x4.max(axis=0, keepdims=True)     # global max pool, this graph
    x5 = jnp.broadcast_to(pooled, (x.shape[0], pooled.shape[1]))
    h = jnp.concatenate([x1, x2, x3, x5], axis=1)
    h = bn(jax.nn.relu(h @ w['m0_w'] + w['m0_b']), w['m0_g'], w['m0_be'])
    h = bn(jax.nn.relu(h @ w['m1_w'] + w['m1_b']), w['m1_g'], w['m1_be'])
    return h @ w['m2_w'] + w['m2_b']


_CACHE = {}

# fixed packing order for the single flat weight upload (one RPC instead of
# ~40 — per-array host->device transfers dominate wall time on this setup)
_W_KEYS = tuple(
    [f"{c}_{p}" for c in ("c1", "c2", "c3")
     for p in ("w0", "b0", "g0", "be0", "w1", "b1", "g1", "be1")]
    + ["l1_w", "l1_b", "l1_g", "l1_be", "m0_w", "m0_b", "m0_g", "m0_be",
       "m1_w", "m1_b", "m1_g", "m1_be", "m2_w", "m2_b"])


def _get_fn(B, shapes):
    import jax
    import jax.numpy as jnp
    from jax.sharding import Mesh, PartitionSpec as P
    from jax.experimental.shard_map import shard_map

    key = ('fn', B, shapes)
    if key not in _CACHE:
        devs = jax.devices()[:N_CORES]
        mesh = Mesh(np.asarray(devs), ('c',))
        sizes = [int(np.prod(s)) for s in shapes]
        offs = np.concatenate([[0], np.cumsum(sizes)]).tolist()

        def body(x, flat):
            w = {k: jnp.reshape(flat[offs[i]:offs[i + 1]], shapes[i])
                 for i, k in enumerate(_W_KEYS)}
            return _forward_sharded(x, w, B)

        _CACHE[key] = jax.jit(shard_map(
            body, mesh=mesh,
            in_specs=(P('c'), P()), out_specs=P('c'), check_rep=False))
    return _CACHE[key]


def _reference_single(x, w, B):
    """Unsharded fallback for batch sizes that don't match the core count."""
    import jax
    import jax.numpy as jnp

    def bn(h, g, b):
        m = h.mean(axis=0, keepdims=True)
        v = jnp.var(h, axis=0, keepdims=True)
        return (h - m) * jax.lax.rsqrt(v + EPS) * g + b

    def edge_conv(xf, w0, b0, g0, be0, w1, b1, g1, be1):
        T, D = xf.shape
        N = T // B
        xb = xf.reshape(B, N, D)
        sq = jnp.sum(xb * xb, axis=-1)
        d2 = sq[:, :, None] + sq[:, None, :] - 2.0 * jnp.einsum(
            'bnd,bmd->bnm', xb, xb)
        _, idx = jax.lax.top_k(-d2, K)
        neigh = jax.vmap(lambda xg, ig: xg[ig])(xb, idx)
        xi = jnp.broadcast_to(xb[:, :, None, :], neigh.shape)
        e = jnp.concatenate([xi, neigh - xi], axis=-1).reshape(T * K, 2 * D)
        h = bn(jax.nn.relu(e @ w0 + b0), g0, be0)
        h = bn(jax.nn.relu(h @ w1 + b1), g1, be1)
        return h.reshape(B, N, K, -1).max(axis=2).reshape(T, -1)

    x1 = edge_conv(x, w['c1_w0'], w['c1_b0'], w['c1_g0'], w['c1_be0'],
                   w['c1_w1'], w['c1_b1'], w['c1_g1'], w['c1_be1'])
    x2 = edge_conv(x1, w['c2_w0'], w['c2_b0'], w['c2_g0'], w['c2_be0'],
                   w['c2_w1'], w['c2_b1'], w['c2_g1'], w['c2_be1'])
    x3 = edge_conv(x2, w['c3_w0'], w['c3_b0'], w['c3_g0'], w['c3_be0'],
                   w['c3_w1'], w['c3_b1'], w['c3_g1'], w['c3_be1'])
    cat = jnp.concatenate([x1, x2, x3], axis=1)
    x4 = bn(jax.nn.relu(cat @ w['l1_w'] + w['l1_b']), w['l1_g'], w['l1_be'])
    N = x.shape[0] // B
    pooled = x4.reshape(B, N, -1).max(axis=1)
    x5 = jnp.repeat(pooled, N, axis=0)
    h = jnp.concatenate([x1, x2, x3, x5], axis=1)
    h = bn(jax.nn.relu(h @ w['m0_w'] + w['m0_b']), w['m0_g'], w['m0_be'])
    h = bn(jax.nn.relu(h @ w['m1_w'] + w['m1_b']), w['m1_g'], w['m1_be'])
    return h @ w['m2_w'] + w['m2_b']


def kernel(**inputs) -> np.ndarray:
    import jax
    import jax.numpy as jnp

    x = np.asarray(inputs['x'], np.float32)
    B = int(inputs['batch_size'])

    if B == N_CORES and x.shape[0] % N_CORES == 0:
        ws = [np.asarray(inputs[k], np.float32) for k in _W_KEYS]
        shapes = tuple(tuple(a.shape) for a in ws)
        flat = np.concatenate([a.ravel() for a in ws])
        fn = _get_fn(B, shapes)
        out = fn(jnp.asarray(x), jnp.asarray(flat))
    else:
        w = {k: jnp.asarray(np.asarray(v)) for k, v in inputs.items()
             if k not in ('x', 'batch_size')}
        from functools import partial
        key = ('single', B)
        if key not in _CACHE:
            _CACHE[key] = jax.jit(partial(_reference_single, B=B))
        out = _CACHE[key](jnp.asarray(x), w)
    return np.asarray(out)   # blocks until ready

